# revision 23
# baseline (speedup 1.0000x reference)
"""Grouped-experts SwiGLU FFN on 8 TRN2 NeuronCores.

Per-expert: out_e = (silu(x_e @ w1_e) * (x_e @ w3_e)) @ w2_e
E=8, T=2048, D=2048, H=4096 (fp32 in/out). Expert-parallel: core e owns
expert e; no cross-core communication.

v2 (vs the 1.40 ms v1, which was PE-bound at 216ns/matmul with ~38us of
idle + 27us of fp32 PE transposes; a GPIO power throttle (81.25% duty)
kicks in ~1ms into the kernel, amplifying every saved us by ~1.23x):
  - x transposes moved off the PE onto the DMA XBAR (fp16
    dma_start_transpose, 14ns per 16x128 tile). Only slabs 0-3 stay on
    the PE (fp32 transposes) as startup filler while the 16MB x load
    streams. xT is laid out [P, TM, KD, 128] so each XBAR write is a
    contiguous per-partition block (non-contiguous XBAR dests corrupt
    on HW; see tile_matmul.py).
  - Two HWDGE queues: x slabs/gLo bounce/readback/out on qSP (nc.sync),
    w1/w3/w2 staging + XBAR transposes on qAct (nc.scalar) so weight
    staging never queues behind the x stream.
  - hm=0 and hm=1 run interleaved at 128-wide (slab-granular) matmul
    groups so the PE always has ready work during the startup DMA
    crunch; tn=1..3 of both run 512-wide back-to-back. hm>=2 is the
    plain steady state.
  - w2's first 12 k-tiles (quarter 0 + half of quarter 1) are
    DMA-prefetched into SBUF during hm=30..31, so phase B's first
    matmul group starts ~2us after phase A's last. The gLo readback
    (8MB) runs on qSP in parallel with w2 staging on qAct.
  - Last output group evicts as 2x256-wide on DVE+ACT concurrently and
    DMAs out on both queues to shorten the drain tail.
"""

import os
import sys
from contextlib import ExitStack

import numpy as np

for _p in ("/opt/trn_rl_repo", "/root/.axon_site/_ro/trn_rl_repo"):
    if os.path.isdir(_p) and _p not in sys.path:
        sys.path.insert(0, _p)

import concourse.bass as bass
import concourse.tile as tile
from concourse import bacc, mybir
from concourse._compat import with_exitstack
from concourse.bass_utils import run_bass_kernel_spmd
from concourse.masks import make_identity

E, T, D, H = 8, 2048, 2048, 4096
P = 128
KD = D // P        # 16 k-tiles over D (mm1/mm3 contraction)
KH = H // P        # 32 k-tiles over H (mm2 contraction)
KLO = KH // 2      # 16: k-tiles of g bounced via DRAM (low half)
HM = H // P        # 32 output-partition tiles of hT
TN = T // 512      # 4 moving chunks of T for mm1/mm3
TM = T // P        # 16 t-blocks (slabs)
DB = 512           # mm2 moving-dim chunk of D
DN = D // DB       # 4
NPE = 4            # slabs transposed on the PE (startup filler)

F32 = mybir.dt.float32
F16 = mybir.dt.float16
# ACT_FN is module-level so sim_test.py can swap in Sigmoid (CoreSim's
# interp has no Silu); hardware always runs the real Silu.
ACT_FN = mybir.ActivationFunctionType.Silu

TRACE = False
LAST_RESULTS = None
_CACHED_NC = None


@with_exitstack
def _swiglu_body(ctx: ExitStack, tc: "tile.TileContext", out, x, w1, w2, w3, gLoD):
    nc = tc.nc

    consts = ctx.enter_context(tc.tile_pool(name="consts", bufs=1))
    bigA = ctx.enter_context(tc.tile_pool(name="bigA", bufs=1))
    bigB = ctx.enter_context(tc.tile_pool(name="bigB", bufs=1))
    psum = ctx.enter_context(tc.tile_pool(name="psum", bufs=8, space="PSUM"))
    # w2 quarter-0 prefetch slots; ctx-level because phase B reads them
    # after the phase-A pools close (pool scopes are strict LIFO).
    w2preP = ctx.enter_context(tc.tile_pool(name="w2pre", bufs=2))

    # xT layout [p, tb, k, t']: slab tb's block is contiguous per
    # partition (XBAR requirement). Shares its SBUF slot with the gLo
    # readback (tag bufs=1).
    xT = bigA.tile([P, TM, KD, P], F16, tag="bigA", name="xT")
    # g k-tiles 16..31 live here, written directly by phase A.
    gHiS = bigB.tile([P, KH - KLO, T], F16, tag="bigB", name="gHiS")

    ident = consts.tile([P, P], F32)
    make_identity(nc, ident[:])
    ident16 = consts.tile([P, P], F16)
    make_identity(nc, ident16[:])

    w1r = w1.rearrange("(k p) h -> p k h", p=P)
    w3r = w3.rearrange("(k p) h -> p k h", p=P)
    w2r = w2.rearrange("(k p) d -> p k d", p=P)

    with tc.tile_pool(name="w13stage", bufs=2) as wstage, \
         tc.tile_pool(name="w13q", bufs=2) as wq, \
         tc.tile_pool(name="silu", bufs=2) as silu, \
         tc.tile_pool(name="gstrip", bufs=3) as gstrip:

        def emit_w13(hm):
            wst1 = wstage.tile([P, KD, P], F32, tag="wst")
            wst3 = wstage.tile([P, KD, P], F32, tag="wst")
            nc.scalar.dma_start(wst1[:], w1r[:, :, hm * P:(hm + 1) * P])
            nc.scalar.dma_start(wst3[:], w3r[:, :, hm * P:(hm + 1) * P])
            wqt = wq.tile([P, 2, KD, P], F16, tag="wq")
            nc.scalar.copy(wqt[:, 0], wst1[:])
            nc.scalar.copy(wqt[:, 1], wst3[:])
            return wqt

        def emit_group(hm, wqt, t0, w):
            # one (hm, [t0, t0+w)) SwiGLU group; w in {128, 512}
            tb0 = t0 // P
            nb = w // P

            def mov(k):
                if nb == 1:
                    return xT[:, tb0, k, :]
                return xT[:, tb0:tb0 + nb, k, :]

            ps1 = psum.tile([P, w], F32, tag="ps")
            ps3 = psum.tile([P, w], F32, tag="ps")
            for k in range(KD):
                nc.tensor.matmul(
                    ps1[:], wqt[:, 0, k, :], mov(k),
                    start=(k == 0), stop=(k == KD - 1),
                )
            for k in range(KD):
                nc.tensor.matmul(
                    ps3[:], wqt[:, 1, k, :], mov(k),
                    start=(k == 0), stop=(k == KD - 1),
                )
            sl = silu.tile([P, w], F32, tag="sl")
            nc.scalar.activation(sl[:], ps1[:], ACT_FN)
            ts_ = slice(t0, t0 + w)
            if hm >= KLO:
                nc.vector.tensor_mul(gHiS[:, hm - KLO, ts_], sl[:], ps3[:])
            else:
                gs = gstrip.tile([P, w], F16, tag="gs")
                nc.vector.tensor_mul(gs[:], sl[:], ps3[:])
                # dispatch on qAct: a gLo store waits on the DVE mul,
                # and on qSP it would head-of-line-block the x slab
                # loads queued behind it (~11us PE stall).
                nc.scalar.dma_start(gLoD[hm * P:(hm + 1) * P, ts_], gs[:])

        with tc.tile_pool(name="xs32", bufs=4) as xs32p, \
             tc.tile_pool(name="xs16", bufs=3) as xs16p:

            # x ingest: loads are emitted one chunk AHEAD of their
            # transposes so qSP is a pure, uninterrupted x stream (the
            # gLo stores dispatch on qAct for the same reason). PE
            # transposes cost zero DMA — the ingest window is
            # DMA-bound, so PE transposes there are nearly free (an
            # XBAR variant was 60us slower from +16MB of DMA). Slabs
            # 0-3 run fp32 (filler while the first slabs land); later
            # slabs sit in PE-bound windows, so cast on DVE first and
            # transpose fp16 at 1 cyc/row instead of 2.
            slab_tiles = {}

            def emit_slab_load(tb):
                for half in (0, 1):
                    xs = xs32p.tile([P, 1024], F32, tag="xs32")
                    nc.sync.dma_start(
                        xs[:],
                        x[tb * P:(tb + 1) * P, half * 1024:(half + 1) * 1024],
                    )
                    slab_tiles[(tb, half)] = xs

            def emit_slab_tps(tb):
                for half in (0, 1):
                    xs = slab_tiles.pop((tb, half))
                    fp16_tp = tb >= NPE
                    if fp16_tp:
                        xh = xs16p.tile([P, 1024], F16, tag="xs16")
                        nc.vector.tensor_copy(xh[:], xs[:])
                    for j in range(8):
                        k = half * 8 + j
                        if fp16_tp:
                            ps = psum.tile([P, P], F16, tag="ps")
                            nc.tensor.transpose(
                                ps[:], xh[:, j * P:(j + 1) * P], ident16[:]
                            )
                        else:
                            ps = psum.tile([P, P], F32, tag="ps")
                            nc.tensor.transpose(
                                ps[:], xs[:, j * P:(j + 1) * P], ident[:]
                            )
                        if k % 2 == 0:
                            nc.vector.tensor_copy(xT[:, tb, k, :], ps[:])
                        else:
                            nc.scalar.copy(xT[:, tb, k, :], ps[:])

            # ---- hm=0,1 interleaved with the x stream: running both
            # per tn-chunk gives the PE ~27us of work per 4-slab DMA
            # window, so slabs always arrive ahead of their groups.
            emit_slab_load(0)
            wqt0 = emit_w13(0)
            emit_slab_load(1)
            wqt1 = emit_w13(1)
            emit_slab_load(2)
            emit_slab_load(3)
            emit_slab_tps(0)
            emit_slab_tps(1)
            emit_slab_tps(2)
            emit_slab_tps(3)
            for tn in range(TN):
                if tn > 0:
                    for tb in range(4 * tn, 4 * tn + 4):
                        emit_slab_tps(tb)
                for tb in range(4 * tn + 4, min(4 * tn + 8, TM)):
                    emit_slab_load(tb)
                emit_group(0, wqt0, tn * 512, 512)
                emit_group(1, wqt1, tn * 512, 512)

        # ---- remaining hm steady state. gHiS writers (hm>=16) run
        # FIRST: dependency tracking is tile-granular, so phase B's
        # first gHiS read waits on the LAST gHiS write — push that
        # ~190us before phase A's end. Prefetch w2 k-tiles 16..23
        # (quarter 0) near the end so phase B starts instantly.
        w2pre_tiles = []
        hm_order = list(range(KLO, HM)) + list(range(2, KLO))
        for pos, hm in enumerate(hm_order):
            wqt = emit_w13(hm)
            if pos == len(hm_order) - 2:
                for k0 in (KLO, KLO + 4):
                    st = w2preP.tile([P, 4, DB], F32, tag="w2pre")
                    nc.scalar.dma_start(st[:], w2r[:, k0:k0 + 4, 0:DB])
                    w2pre_tiles.append(st)
            for tn in range(TN):
                emit_group(hm, wqt, tn * 512, 512)

    # ---- Phase B: out[T,D] = g @ w2, k over H; FD=512; k-order hi->lo.
    ks_order = list(range(KLO, KH)) + list(range(0, KLO))
    with tc.tile_pool(name="w2stage", bufs=2) as w2stage, \
         tc.tile_pool(name="w2q", bufs=4) as w2q, \
         tc.tile_pool(name="oevict", bufs=4) as oevict:

        # gLo readback into xT's slot, on qSP (parallel with w2 on qAct).
        gLoS = bigA.tile([P, KLO, T], F16, tag="bigA", name="gLoS")
        for k in range(KLO):
            nc.sync.dma_start(gLoS[:, k, :], gLoD[k * P:(k + 1) * P, :])

        def emit_w2quarters(dn):
            ds_ = slice(dn * DB, (dn + 1) * DB)
            quarters = []
            for q in range(4):
                wh = w2q.tile([P, 8, DB], F16, tag="w2q")
                quarters.append(wh)
                for s in range(2):
                    idx = q * 8 + s * 4
                    k0 = ks_order[idx]
                    if dn == 0 and idx < 8:
                        st = w2pre_tiles[idx // 4]
                    else:
                        st = w2stage.tile([P, 4, DB], F32, tag="w2s")
                        nc.scalar.dma_start(st[:], w2r[:, k0:k0 + 4, ds_])
                    # split casts across ACT/DVE so the dn=0 bunch
                    # drains 2x faster at the A->B transition
                    if s == 0:
                        nc.scalar.copy(wh[:, s * 4:(s + 1) * 4, :], st[:])
                    else:
                        nc.vector.tensor_copy(wh[:, s * 4:(s + 1) * 4, :], st[:])
            return quarters

        def gblk(k):
            return gLoS[:, k, :] if k < KLO else gHiS[:, k - KLO, :]

        for dn in range(DN):
            ds_ = slice(dn * DB, (dn + 1) * DB)
            quarters = emit_w2quarters(dn)
            for tg in range(4):
                last = (dn == DN - 1) and (tg == 3)
                pss = [
                    psum.tile([P, DB], F32, tag="ps", name=f"pso_{dn}_{tg}_{i}")
                    for i in range(4)
                ]
                for ki, k in enumerate(ks_order):
                    wmv = quarters[ki // 8][:, ki % 8, :]
                    for i in range(4):
                        tm = tg * 4 + i
                        nc.tensor.matmul(
                            pss[i][:],
                            gblk(k)[:, tm * P:(tm + 1) * P],
                            wmv,
                            start=(ki == 0), stop=(ki == KH - 1),
                        )
                for i in range(4):
                    tm = tg * 4 + i
                    ev = oevict.tile([P, DB], F32, tag="ev")
                    if last:
                        # split across both engines + both queues to
                        # shorten the end-of-kernel drain
                        nc.vector.tensor_copy(ev[:, 0:DB // 2], pss[i][:, 0:DB // 2])
                        nc.scalar.copy(ev[:, DB // 2:], pss[i][:, DB // 2:])
                        nc.sync.dma_start(
                            out[tm * P:(tm + 1) * P,
                                dn * DB:dn * DB + DB // 2],
                            ev[:, 0:DB // 2],
                        )
                        nc.scalar.dma_start(
                            out[tm * P:(tm + 1) * P,
                                dn * DB + DB // 2:(dn + 1) * DB],
                            ev[:, DB // 2:],
                        )
                    else:
                        if i % 2 == 0:
                            nc.vector.tensor_copy(ev[:], pss[i][:])
                        else:
                            nc.scalar.copy(ev[:], pss[i][:])
                        if i % 2 == 0:
                            nc.sync.dma_start(out[tm * P:(tm + 1) * P, ds_], ev[:])
                        else:
                            nc.scalar.dma_start(out[tm * P:(tm + 1) * P, ds_], ev[:])


def _build(num_devices=E):
    nc = bacc.Bacc("TRN2", debug=False, num_devices=num_devices)
    x = nc.dram_tensor("x", (T, D), F32, kind="ExternalInput").ap()
    w1 = nc.dram_tensor("w1", (D, H), F32, kind="ExternalInput").ap()
    w2 = nc.dram_tensor("w2", (H, D), F32, kind="ExternalInput").ap()
    w3 = nc.dram_tensor("w3", (D, H), F32, kind="ExternalInput").ap()
    out = nc.dram_tensor("out", (T, D), F32, kind="ExternalOutput").ap()
    gLoD = nc.dram_tensor("gLoD", (KLO * P, T), F16, kind="Internal").ap()
    with tile.TileContext(nc) as tc:
        _swiglu_body(tc, out, x, w1, w2, w3, gLoD)
    nc.compile()
    return nc


def _get_nc():
    global _CACHED_NC
    if _CACHED_NC is None:
        _CACHED_NC = _build()
    return _CACHED_NC


def kernel(x, w1, w2, w3):
    global LAST_RESULTS
    x = np.ascontiguousarray(np.asarray(x, dtype=np.float32))
    w1 = np.ascontiguousarray(np.asarray(w1, dtype=np.float32))
    w2 = np.ascontiguousarray(np.asarray(w2, dtype=np.float32))
    w3 = np.ascontiguousarray(np.asarray(w3, dtype=np.float32))
    assert x.shape == (E, T, D), x.shape

    nc = _get_nc()
    in_maps = [
        {"x": x[e], "w1": w1[e], "w2": w2[e], "w3": w3[e]} for e in range(E)
    ]
    res = run_bass_kernel_spmd(
        nc, in_maps, core_ids=list(range(E)), trace=TRACE
    )
    LAST_RESULTS = res
    return np.stack([res.results[e]["out"] for e in range(E)], axis=0)


# revision 29
# speedup vs baseline: 1.0060x; 1.0060x over previous
"""Grouped-experts SwiGLU FFN on 8 TRN2 NeuronCores.

Per-expert: out_e = (silu(x_e @ w1_e) * (x_e @ w3_e)) @ w2_e
E=8, T=2048, D=2048, H=4096 (fp32 in/out). Expert-parallel: core e owns
expert e; no cross-core communication.

v2 (vs the 1.40 ms v1, which was PE-bound at 216ns/matmul with ~38us of
idle + 27us of fp32 PE transposes; a GPIO power throttle (81.25% duty)
kicks in ~1ms into the kernel, amplifying every saved us by ~1.23x):
  - x transposes moved off the PE onto the DMA XBAR (fp16
    dma_start_transpose, 14ns per 16x128 tile). Only slabs 0-3 stay on
    the PE (fp32 transposes) as startup filler while the 16MB x load
    streams. xT is laid out [P, TM, KD, 128] so each XBAR write is a
    contiguous per-partition block (non-contiguous XBAR dests corrupt
    on HW; see tile_matmul.py).
  - Two HWDGE queues: x slabs/gLo bounce/readback/out on qSP (nc.sync),
    w1/w3/w2 staging + XBAR transposes on qAct (nc.scalar) so weight
    staging never queues behind the x stream.
  - hm=0 and hm=1 run interleaved at 128-wide (slab-granular) matmul
    groups so the PE always has ready work during the startup DMA
    crunch; tn=1..3 of both run 512-wide back-to-back. hm>=2 is the
    plain steady state.
  - w2's first 12 k-tiles (quarter 0 + half of quarter 1) are
    DMA-prefetched into SBUF during hm=30..31, so phase B's first
    matmul group starts ~2us after phase A's last. The gLo readback
    (8MB) runs on qSP in parallel with w2 staging on qAct.
  - Last output group evicts as 2x256-wide on DVE+ACT concurrently and
    DMAs out on both queues to shorten the drain tail.
"""

import os
import sys
from contextlib import ExitStack

import numpy as np

for _p in ("/opt/trn_rl_repo", "/root/.axon_site/_ro/trn_rl_repo"):
    if os.path.isdir(_p) and _p not in sys.path:
        sys.path.insert(0, _p)

import concourse.bass as bass
import concourse.tile as tile
from concourse import bacc, mybir
from concourse._compat import with_exitstack
from concourse.bass_utils import run_bass_kernel_spmd
from concourse.masks import make_identity

E, T, D, H = 8, 2048, 2048, 4096
P = 128
KD = D // P        # 16 k-tiles over D (mm1/mm3 contraction)
KH = H // P        # 32 k-tiles over H (mm2 contraction)
KLO = KH // 2      # 16: k-tiles of g bounced via DRAM (low half)
HM = H // P        # 32 output-partition tiles of hT
TN = T // 512      # 4 moving chunks of T for mm1/mm3
TM = T // P        # 16 t-blocks (slabs)
DB = 512           # mm2 moving-dim chunk of D
DN = D // DB       # 4
NPE = 4            # slabs transposed on the PE (startup filler)

F32 = mybir.dt.float32
F16 = mybir.dt.float16
# ACT_FN is module-level so sim_test.py can swap in Sigmoid (CoreSim's
# interp has no Silu); hardware always runs the real Silu.
ACT_FN = mybir.ActivationFunctionType.Silu

TRACE = False
LAST_RESULTS = None
_CACHED_NC = None


@with_exitstack
def _swiglu_body(ctx: ExitStack, tc: "tile.TileContext", out, x, w1, w2, w3, gLoD):
    nc = tc.nc

    consts = ctx.enter_context(tc.tile_pool(name="consts", bufs=1))
    bigA = ctx.enter_context(tc.tile_pool(name="bigA", bufs=1))
    bigB = ctx.enter_context(tc.tile_pool(name="bigB", bufs=1))
    psum = ctx.enter_context(tc.tile_pool(name="psum", bufs=8, space="PSUM"))
    # w2 quarter-0, staged AND cast to fp16 during late phase A;
    # ctx-level because phase B reads it after the phase-A pools close
    # (pool scopes are strict LIFO).
    w2preP = ctx.enter_context(tc.tile_pool(name="w2pre", bufs=1))
    wh0P = w2preP.tile([P, 8, DB], F16, tag="wh0", name="wh0P")

    # xT layout [p, tb, k, t']: slab tb's block is contiguous per
    # partition (XBAR requirement). Shares its SBUF slot with the gLo
    # readback (tag bufs=1).
    xT = bigA.tile([P, TM, KD, P], F16, tag="bigA", name="xT")
    # g k-tiles 16..31 live here, written directly by phase A.
    gHiS = bigB.tile([P, KH - KLO, T], F16, tag="bigB", name="gHiS")

    ident = consts.tile([P, P], F32)
    make_identity(nc, ident[:])
    ident16 = consts.tile([P, P], F16)
    make_identity(nc, ident16[:])

    w1r = w1.rearrange("(k p) h -> p k h", p=P)
    w3r = w3.rearrange("(k p) h -> p k h", p=P)
    w2r = w2.rearrange("(k p) d -> p k d", p=P)

    with tc.tile_pool(name="w13stage", bufs=3) as wstage, \
         tc.tile_pool(name="w13q", bufs=2) as wq, \
         tc.tile_pool(name="silu", bufs=2) as silu, \
         tc.tile_pool(name="gstrip", bufs=2) as gstrip:

        def emit_w13(hm):
            wst1 = wstage.tile([P, KD, P], F32, tag="wst")
            wst3 = wstage.tile([P, KD, P], F32, tag="wst")
            nc.scalar.dma_start(wst1[:], w1r[:, :, hm * P:(hm + 1) * P])
            nc.scalar.dma_start(wst3[:], w3r[:, :, hm * P:(hm + 1) * P])
            wqt = wq.tile([P, 2, KD, P], F16, tag="wq")
            nc.scalar.copy(wqt[:, 0], wst1[:])
            nc.scalar.copy(wqt[:, 1], wst3[:])
            return wqt

        def emit_group(hm, wqt, t0, w):
            # one (hm, [t0, t0+w)) SwiGLU group; w in {128, 512}
            tb0 = t0 // P
            nb = w // P

            def mov(k):
                if nb == 1:
                    return xT[:, tb0, k, :]
                return xT[:, tb0:tb0 + nb, k, :]

            ps1 = psum.tile([P, w], F32, tag="ps")
            ps3 = psum.tile([P, w], F32, tag="ps")
            for k in range(KD):
                nc.tensor.matmul(
                    ps1[:], wqt[:, 0, k, :], mov(k),
                    start=(k == 0), stop=(k == KD - 1),
                )
            for k in range(KD):
                nc.tensor.matmul(
                    ps3[:], wqt[:, 1, k, :], mov(k),
                    start=(k == 0), stop=(k == KD - 1),
                )
            sl = silu.tile([P, w], F32, tag="sl")
            nc.scalar.activation(sl[:], ps1[:], ACT_FN)
            ts_ = slice(t0, t0 + w)
            if hm >= KLO:
                nc.vector.tensor_mul(gHiS[:, hm - KLO, ts_], sl[:], ps3[:])
            else:
                gs = gstrip.tile([P, w], F16, tag="gs")
                nc.vector.tensor_mul(gs[:], sl[:], ps3[:])
                # dispatch on qAct: a gLo store waits on the DVE mul,
                # and on qSP it would head-of-line-block the x slab
                # loads queued behind it (~11us PE stall).
                nc.scalar.dma_start(gLoD[hm * P:(hm + 1) * P, ts_], gs[:])

        with tc.tile_pool(name="xs32", bufs=5) as xs32p, \
             tc.tile_pool(name="xs16", bufs=2) as xs16p:

            # x ingest: loads are emitted one chunk AHEAD of their
            # transposes so qSP is a pure, uninterrupted x stream (the
            # gLo stores dispatch on qAct for the same reason). PE
            # transposes cost zero DMA — the ingest window is
            # DMA-bound, so PE transposes there are nearly free (an
            # XBAR variant was 60us slower from +16MB of DMA). Slabs
            # 0-3 run fp32 (filler while the first slabs land); later
            # slabs sit in PE-bound windows, so cast on DVE first and
            # transpose fp16 at 1 cyc/row instead of 2.
            slab_tiles = {}

            def emit_slab_load(tb):
                for half in (0, 1):
                    xs = xs32p.tile([P, 1024], F32, tag="xs32")
                    nc.sync.dma_start(
                        xs[:],
                        x[tb * P:(tb + 1) * P, half * 1024:(half + 1) * 1024],
                    )
                    slab_tiles[(tb, half)] = xs

            def emit_slab_tps(tb):
                for half in (0, 1):
                    xs = slab_tiles.pop((tb, half))
                    fp16_tp = tb >= NPE
                    if fp16_tp:
                        xh = xs16p.tile([P, 1024], F16, tag="xs16")
                        nc.vector.tensor_copy(xh[:], xs[:])
                    for j in range(8):
                        k = half * 8 + j
                        if fp16_tp:
                            ps = psum.tile([P, P], F16, tag="ps")
                            nc.tensor.transpose(
                                ps[:], xh[:, j * P:(j + 1) * P], ident16[:]
                            )
                        else:
                            ps = psum.tile([P, P], F32, tag="ps")
                            nc.tensor.transpose(
                                ps[:], xs[:, j * P:(j + 1) * P], ident[:]
                            )
                        if k % 2 == 0:
                            nc.vector.tensor_copy(xT[:, tb, k, :], ps[:])
                        else:
                            nc.scalar.copy(xT[:, tb, k, :], ps[:])

            # ---- hm=0,1 interleaved with the x stream: running both
            # per tn-chunk gives the PE ~27us of work per 4-slab DMA
            # window, so slabs always arrive ahead of their groups.
            emit_slab_load(0)
            wqt0 = emit_w13(0)
            emit_slab_load(1)
            wqt1 = emit_w13(1)
            emit_slab_load(2)
            emit_slab_load(3)
            emit_slab_tps(0)
            emit_slab_tps(1)
            emit_slab_tps(2)
            emit_slab_tps(3)
            for tn in range(TN):
                if tn > 0:
                    for tb in range(4 * tn, 4 * tn + 4):
                        emit_slab_tps(tb)
                for tb in range(4 * tn + 4, min(4 * tn + 8, TM)):
                    emit_slab_load(tb)
                emit_group(0, wqt0, tn * 512, 512)
                emit_group(1, wqt1, tn * 512, 512)

        # ---- remaining hm steady state. gHiS writers (hm>=16) run
        # FIRST: dependency tracking is tile-granular, so phase B's
        # first gHiS read waits on the LAST gHiS write — push that
        # ~190us before phase A's end. Near the end, stage w2's
        # quarter 0 (k-tiles 16..23) and cast it into the ctx-level
        # wh0P, so phase B's first ~7us of matmuls depend on nothing
        # allocated after phase A.
        hm_order = list(range(KLO, HM)) + list(range(2, KLO))
        for pos, hm in enumerate(hm_order):
            wqt = emit_w13(hm)
            if pos == len(hm_order) - 2:
                for s, k0 in enumerate((KLO, KLO + 4)):
                    st = wstage.tile([P, 4, DB], F32, tag="wst")
                    nc.scalar.dma_start(st[:], w2r[:, k0:k0 + 4, 0:DB])
                    if s == 0:
                        nc.scalar.copy(wh0P[:, 0:4, :], st[:])
                    else:
                        nc.vector.tensor_copy(wh0P[:, 4:8, :], st[:])
            for tn in range(TN):
                emit_group(hm, wqt, tn * 512, 512)

    # ---- Phase B: out[T,D] = g @ w2, k over H; FD=512; k-order hi->lo.
    ks_order = list(range(KLO, KH)) + list(range(0, KLO))
    with tc.tile_pool(name="w2stage", bufs=2) as w2stage, \
         tc.tile_pool(name="w2q", bufs=4) as w2q, \
         tc.tile_pool(name="oevict", bufs=4) as oevict:

        def emit_w2quarters(dn):
            ds_ = slice(dn * DB, (dn + 1) * DB)
            quarters = []
            for q in range(4):
                if dn == 0 and q == 0:
                    # prefetched + cast during phase A (ctx-level tile)
                    quarters.append(wh0P)
                    continue
                wh = w2q.tile([P, 8, DB], F16, tag="w2q")
                quarters.append(wh)
                for s in range(2):
                    idx = q * 8 + s * 4
                    k0 = ks_order[idx]
                    st = w2stage.tile([P, 4, DB], F32, tag="w2s")
                    nc.scalar.dma_start(st[:], w2r[:, k0:k0 + 4, ds_])
                    # split casts across ACT/DVE so the dn=0 bunch
                    # drains 2x faster at the A->B transition
                    if s == 0:
                        nc.scalar.copy(wh[:, s * 4:(s + 1) * 4, :], st[:])
                    else:
                        nc.vector.tensor_copy(wh[:, s * 4:(s + 1) * 4, :], st[:])
            return quarters

        # dn=0's quarters are emitted BEFORE the gLo readback: the
        # allocator coalesces pool-space-reuse waits into queue
        # counters by emission order, so quarters emitted after the
        # strips would wait on strip DMA completions (~8us stall).
        quarters0 = emit_w2quarters(0)

        # gLo readback into xT's slot, on qSP (parallel with w2 on qAct).
        gLoS = bigA.tile([P, KLO, T], F16, tag="bigA", name="gLoS")
        for k in range(KLO):
            nc.sync.dma_start(gLoS[:, k, :], gLoD[k * P:(k + 1) * P, :])

        def gblk(k):
            return gLoS[:, k, :] if k < KLO else gHiS[:, k - KLO, :]

        for dn in range(DN):
            ds_ = slice(dn * DB, (dn + 1) * DB)
            quarters = quarters0 if dn == 0 else emit_w2quarters(dn)
            for tg in range(4):
                last = (dn == DN - 1) and (tg == 3)
                pss = [
                    psum.tile([P, DB], F32, tag="ps", name=f"pso_{dn}_{tg}_{i}")
                    for i in range(4)
                ]
                for ki, k in enumerate(ks_order):
                    wmv = quarters[ki // 8][:, ki % 8, :]
                    for i in range(4):
                        tm = tg * 4 + i
                        nc.tensor.matmul(
                            pss[i][:],
                            gblk(k)[:, tm * P:(tm + 1) * P],
                            wmv,
                            start=(ki == 0), stop=(ki == KH - 1),
                        )
                for i in range(4):
                    tm = tg * 4 + i
                    ev = oevict.tile([P, DB], F32, tag="ev")
                    if last:
                        # split across both engines + both queues to
                        # shorten the end-of-kernel drain
                        nc.vector.tensor_copy(ev[:, 0:DB // 2], pss[i][:, 0:DB // 2])
                        nc.scalar.copy(ev[:, DB // 2:], pss[i][:, DB // 2:])
                        nc.sync.dma_start(
                            out[tm * P:(tm + 1) * P,
                                dn * DB:dn * DB + DB // 2],
                            ev[:, 0:DB // 2],
                        )
                        nc.scalar.dma_start(
                            out[tm * P:(tm + 1) * P,
                                dn * DB + DB // 2:(dn + 1) * DB],
                            ev[:, DB // 2:],
                        )
                    else:
                        if i % 2 == 0:
                            nc.vector.tensor_copy(ev[:], pss[i][:])
                        else:
                            nc.scalar.copy(ev[:], pss[i][:])
                        if i % 2 == 0:
                            nc.sync.dma_start(out[tm * P:(tm + 1) * P, ds_], ev[:])
                        else:
                            nc.scalar.dma_start(out[tm * P:(tm + 1) * P, ds_], ev[:])


def _build(num_devices=E):
    nc = bacc.Bacc("TRN2", debug=False, num_devices=num_devices)
    x = nc.dram_tensor("x", (T, D), F32, kind="ExternalInput").ap()
    w1 = nc.dram_tensor("w1", (D, H), F32, kind="ExternalInput").ap()
    w2 = nc.dram_tensor("w2", (H, D), F32, kind="ExternalInput").ap()
    w3 = nc.dram_tensor("w3", (D, H), F32, kind="ExternalInput").ap()
    out = nc.dram_tensor("out", (T, D), F32, kind="ExternalOutput").ap()
    gLoD = nc.dram_tensor("gLoD", (KLO * P, T), F16, kind="Internal").ap()
    with tile.TileContext(nc) as tc:
        _swiglu_body(tc, out, x, w1, w2, w3, gLoD)
    nc.compile()
    return nc


def _get_nc():
    global _CACHED_NC
    if _CACHED_NC is None:
        _CACHED_NC = _build()
    return _CACHED_NC


def kernel(x, w1, w2, w3):
    global LAST_RESULTS
    x = np.ascontiguousarray(np.asarray(x, dtype=np.float32))
    w1 = np.ascontiguousarray(np.asarray(w1, dtype=np.float32))
    w2 = np.ascontiguousarray(np.asarray(w2, dtype=np.float32))
    w3 = np.ascontiguousarray(np.asarray(w3, dtype=np.float32))
    assert x.shape == (E, T, D), x.shape

    nc = _get_nc()
    in_maps = [
        {"x": x[e], "w1": w1[e], "w2": w2[e], "w3": w3[e]} for e in range(E)
    ]
    res = run_bass_kernel_spmd(
        nc, in_maps, core_ids=list(range(E)), trace=TRACE
    )
    LAST_RESULTS = res
    return np.stack([res.results[e]["out"] for e in range(E)], axis=0)


# revision 32
# speedup vs baseline: 1.0161x; 1.0100x over previous
"""Grouped-experts SwiGLU FFN on 8 TRN2 NeuronCores.

Per-expert: out_e = (silu(x_e @ w1_e) * (x_e @ w3_e)) @ w2_e
E=8, T=2048, D=2048, H=4096 (fp32 in/out). Expert-parallel: core e owns
expert e; no cross-core communication.

v2 (vs the 1.40 ms v1, which was PE-bound at 216ns/matmul with ~38us of
idle + 27us of fp32 PE transposes; a GPIO power throttle (81.25% duty)
kicks in ~1ms into the kernel, amplifying every saved us by ~1.23x):
  - x transposes moved off the PE onto the DMA XBAR (fp16
    dma_start_transpose, 14ns per 16x128 tile). Only slabs 0-3 stay on
    the PE (fp32 transposes) as startup filler while the 16MB x load
    streams. xT is laid out [P, TM, KD, 128] so each XBAR write is a
    contiguous per-partition block (non-contiguous XBAR dests corrupt
    on HW; see tile_matmul.py).
  - Two HWDGE queues: x slabs/gLo bounce/readback/out on qSP (nc.sync),
    w1/w3/w2 staging + XBAR transposes on qAct (nc.scalar) so weight
    staging never queues behind the x stream.
  - hm=0 and hm=1 run interleaved at 128-wide (slab-granular) matmul
    groups so the PE always has ready work during the startup DMA
    crunch; tn=1..3 of both run 512-wide back-to-back. hm>=2 is the
    plain steady state.
  - w2's first 12 k-tiles (quarter 0 + half of quarter 1) are
    DMA-prefetched into SBUF during hm=30..31, so phase B's first
    matmul group starts ~2us after phase A's last. The gLo readback
    (8MB) runs on qSP in parallel with w2 staging on qAct.
  - Last output group evicts as 2x256-wide on DVE+ACT concurrently and
    DMAs out on both queues to shorten the drain tail.
"""

import os
import sys
from contextlib import ExitStack

import numpy as np

for _p in ("/opt/trn_rl_repo", "/root/.axon_site/_ro/trn_rl_repo"):
    if os.path.isdir(_p) and _p not in sys.path:
        sys.path.insert(0, _p)

import concourse.bass as bass
import concourse.tile as tile
from concourse import bacc, mybir
from concourse._compat import with_exitstack
from concourse.bass_utils import run_bass_kernel_spmd
from concourse.masks import make_identity

E, T, D, H = 8, 2048, 2048, 4096
P = 128
KD = D // P        # 16 k-tiles over D (mm1/mm3 contraction)
KH = H // P        # 32 k-tiles over H (mm2 contraction)
KLO = KH // 2      # 16: k-tiles of g bounced via DRAM (low half)
HM = H // P        # 32 output-partition tiles of hT
TN = T // 512      # 4 moving chunks of T for mm1/mm3
TM = T // P        # 16 t-blocks (slabs)
DB = 512           # mm2 moving-dim chunk of D
DN = D // DB       # 4
NPE = 4            # slabs transposed on the PE (startup filler)

F32 = mybir.dt.float32
F16 = mybir.dt.float16
# ACT_FN is module-level so sim_test.py can swap in Sigmoid (CoreSim's
# interp has no Silu); hardware always runs the real Silu.
ACT_FN = mybir.ActivationFunctionType.Silu

TRACE = False
LAST_RESULTS = None
_CACHED_NC = None


@with_exitstack
def _swiglu_body(ctx: ExitStack, tc: "tile.TileContext", out, x, w1, w2, w3, gLoD):
    nc = tc.nc

    consts = ctx.enter_context(tc.tile_pool(name="consts", bufs=1))
    bigA = ctx.enter_context(tc.tile_pool(name="bigA", bufs=1))
    bigB = ctx.enter_context(tc.tile_pool(name="bigB", bufs=1))
    psum = ctx.enter_context(tc.tile_pool(name="psum", bufs=8, space="PSUM"))
    # w2 quarter-0, staged AND cast to fp16 during late phase A;
    # ctx-level because phase B reads it after the phase-A pools close
    # (pool scopes are strict LIFO).
    w2preP = ctx.enter_context(tc.tile_pool(name="w2pre", bufs=1))
    wh0P = w2preP.tile([P, 8, DB], F16, tag="wh0", name="wh0P")

    # xT layout [p, tb, k, t']: slab tb's block is contiguous per
    # partition (XBAR requirement). Shares its SBUF slot with the gLo
    # readback (tag bufs=1).
    xT = bigA.tile([P, TM, KD, P], F16, tag="bigA", name="xT")
    # g k-tiles 16..31 live here, written directly by phase A.
    gHiS = bigB.tile([P, KH - KLO, T], F16, tag="bigB", name="gHiS")

    ident = consts.tile([P, P], F32)
    make_identity(nc, ident[:])
    ident16 = consts.tile([P, P], F16)
    make_identity(nc, ident16[:])

    w1r = w1.rearrange("(k p) h -> p k h", p=P)
    w3r = w3.rearrange("(k p) h -> p k h", p=P)
    w2r = w2.rearrange("(k p) d -> p k d", p=P)

    with tc.tile_pool(name="w13stage", bufs=3) as wstage, \
         tc.tile_pool(name="w13q", bufs=2) as wq, \
         tc.tile_pool(name="silu", bufs=2) as silu, \
         tc.tile_pool(name="gstrip", bufs=2) as gstrip:

        def emit_w13(hm):
            wst1 = wstage.tile([P, KD, P], F32, tag="wst")
            wst3 = wstage.tile([P, KD, P], F32, tag="wst")
            nc.scalar.dma_start(wst1[:], w1r[:, :, hm * P:(hm + 1) * P])
            nc.scalar.dma_start(wst3[:], w3r[:, :, hm * P:(hm + 1) * P])
            wqt = wq.tile([P, 2, KD, P], F16, tag="wq")
            # split the two casts across ACT/DVE (ACT is the startup
            # bottleneck: dispatches + evictions + silu)
            nc.scalar.copy(wqt[:, 0], wst1[:])
            nc.vector.tensor_copy(wqt[:, 1], wst3[:])
            return wqt

        def emit_group(hm, wqt, t0, w):
            # one (hm, [t0, t0+w)) SwiGLU group; w in {128, 512}
            tb0 = t0 // P
            nb = w // P

            def mov(k):
                if nb == 1:
                    return xT[:, tb0, k, :]
                return xT[:, tb0:tb0 + nb, k, :]

            ps1 = psum.tile([P, w], F32, tag="ps")
            ps3 = psum.tile([P, w], F32, tag="ps")
            for k in range(KD):
                nc.tensor.matmul(
                    ps1[:], wqt[:, 0, k, :], mov(k),
                    start=(k == 0), stop=(k == KD - 1),
                )
            for k in range(KD):
                nc.tensor.matmul(
                    ps3[:], wqt[:, 1, k, :], mov(k),
                    start=(k == 0), stop=(k == KD - 1),
                )
            sl = silu.tile([P, w], F32, tag="sl")
            nc.scalar.activation(sl[:], ps1[:], ACT_FN)
            ts_ = slice(t0, t0 + w)
            if hm >= KLO:
                nc.vector.tensor_mul(gHiS[:, hm - KLO, ts_], sl[:], ps3[:])
            else:
                gs = gstrip.tile([P, w], F16, tag="gs")
                nc.vector.tensor_mul(gs[:], sl[:], ps3[:])
                # dispatch on qAct: a gLo store waits on the DVE mul,
                # and on qSP it would head-of-line-block the x slab
                # loads queued behind it (~11us PE stall).
                nc.scalar.dma_start(gLoD[hm * P:(hm + 1) * P, ts_], gs[:])

        with tc.tile_pool(name="xs32", bufs=5) as xs32p, \
             tc.tile_pool(name="xs16", bufs=2) as xs16p:

            # x ingest: loads are emitted one chunk AHEAD of their
            # transposes so qSP is a pure, uninterrupted x stream (the
            # gLo stores dispatch on qAct for the same reason). PE
            # transposes cost zero DMA — the ingest window is
            # DMA-bound, so PE transposes there are nearly free (an
            # XBAR variant was 60us slower from +16MB of DMA). Slabs
            # 0-3 run fp32 (filler while the first slabs land); later
            # slabs sit in PE-bound windows, so cast on DVE first and
            # transpose fp16 at 1 cyc/row instead of 2.
            slab_tiles = {}

            def emit_slab_load(tb):
                for half in (0, 1):
                    xs = xs32p.tile([P, 1024], F32, tag="xs32")
                    nc.sync.dma_start(
                        xs[:],
                        x[tb * P:(tb + 1) * P, half * 1024:(half + 1) * 1024],
                    )
                    slab_tiles[(tb, half)] = xs

            def emit_slab_tps(tb):
                # 4 transposes batch into one [P,512] PSUM bank and
                # evict with a single copy: 64 evictions instead of
                # 256 — the per-op fixed cost was saturating ACT/DVE
                # during the startup window.
                for half in (0, 1):
                    xs = slab_tiles.pop((tb, half))
                    fp16_tp = tb >= NPE
                    if fp16_tp:
                        xh = xs16p.tile([P, 1024], F16, tag="xs16")
                        nc.vector.tensor_copy(xh[:], xs[:])
                    for quad in range(2):
                        dt = F16 if fp16_tp else F32
                        ps = psum.tile([P, 4 * P], dt, tag="ps")
                        for j4 in range(4):
                            j = quad * 4 + j4
                            src = (xh if fp16_tp else xs)[:, j * P:(j + 1) * P]
                            idn = ident16 if fp16_tp else ident
                            nc.tensor.transpose(
                                ps[:, j4 * P:(j4 + 1) * P], src, idn[:]
                            )
                        k = half * 8 + quad * 4
                        if (half + quad) % 2 == 0:
                            nc.vector.tensor_copy(xT[:, tb, k:k + 4, :], ps[:])
                        else:
                            nc.scalar.copy(xT[:, tb, k:k + 4, :], ps[:])

            # ---- hm=0,1 interleaved with the x stream: running both
            # per tn-chunk gives the PE ~27us of work per 4-slab DMA
            # window, so slabs always arrive ahead of their groups.
            emit_slab_load(0)
            wqt0 = emit_w13(0)
            emit_slab_load(1)
            wqt1 = emit_w13(1)
            emit_slab_load(2)
            emit_slab_load(3)
            emit_slab_tps(0)
            emit_slab_tps(1)
            emit_slab_tps(2)
            emit_slab_tps(3)
            for tn in range(TN):
                if tn > 0:
                    for tb in range(4 * tn, 4 * tn + 4):
                        emit_slab_tps(tb)
                for tb in range(4 * tn + 4, min(4 * tn + 8, TM)):
                    emit_slab_load(tb)
                emit_group(0, wqt0, tn * 512, 512)
                emit_group(1, wqt1, tn * 512, 512)

        # ---- remaining hm steady state. gHiS writers (hm>=16) run
        # FIRST: dependency tracking is tile-granular, so phase B's
        # first gHiS read waits on the LAST gHiS write — push that
        # ~190us before phase A's end. Near the end, stage w2's
        # quarter 0 (k-tiles 16..23) and cast it into the ctx-level
        # wh0P, so phase B's first ~7us of matmuls depend on nothing
        # allocated after phase A.
        hm_order = list(range(KLO, HM)) + list(range(2, KLO))
        for pos, hm in enumerate(hm_order):
            wqt = emit_w13(hm)
            if pos == len(hm_order) - 2:
                for s, k0 in enumerate((KLO, KLO + 4)):
                    st = wstage.tile([P, 4, DB], F32, tag="wst")
                    nc.scalar.dma_start(st[:], w2r[:, k0:k0 + 4, 0:DB])
                    if s == 0:
                        nc.scalar.copy(wh0P[:, 0:4, :], st[:])
                    else:
                        nc.vector.tensor_copy(wh0P[:, 4:8, :], st[:])
            for tn in range(TN):
                emit_group(hm, wqt, tn * 512, 512)

    # ---- Phase B: out[T,D] = g @ w2, k over H; FD=512; k-order hi->lo.
    ks_order = list(range(KLO, KH)) + list(range(0, KLO))
    with tc.tile_pool(name="w2stage", bufs=2) as w2stage, \
         tc.tile_pool(name="w2q", bufs=4) as w2q, \
         tc.tile_pool(name="oevict", bufs=4) as oevict:

        def emit_w2quarters(dn):
            ds_ = slice(dn * DB, (dn + 1) * DB)
            quarters = []
            for q in range(4):
                if dn == 0 and q == 0:
                    # prefetched + cast during phase A (ctx-level tile)
                    quarters.append(wh0P)
                    continue
                wh = w2q.tile([P, 8, DB], F16, tag="w2q")
                quarters.append(wh)
                for s in range(2):
                    idx = q * 8 + s * 4
                    k0 = ks_order[idx]
                    st = w2stage.tile([P, 4, DB], F32, tag="w2s")
                    nc.scalar.dma_start(st[:], w2r[:, k0:k0 + 4, ds_])
                    # split casts across ACT/DVE so the dn=0 bunch
                    # drains 2x faster at the A->B transition
                    if s == 0:
                        nc.scalar.copy(wh[:, s * 4:(s + 1) * 4, :], st[:])
                    else:
                        nc.vector.tensor_copy(wh[:, s * 4:(s + 1) * 4, :], st[:])
            return quarters

        # dn=0's quarters are emitted BEFORE the gLo readback: the
        # allocator coalesces pool-space-reuse waits into queue
        # counters by emission order, so quarters emitted after the
        # strips would wait on strip DMA completions (~8us stall).
        quarters0 = emit_w2quarters(0)

        # gLo readback into xT's slot, on qSP (parallel with w2 on qAct).
        gLoS = bigA.tile([P, KLO, T], F16, tag="bigA", name="gLoS")
        for k in range(KLO):
            nc.sync.dma_start(gLoS[:, k, :], gLoD[k * P:(k + 1) * P, :])

        def gblk(k):
            return gLoS[:, k, :] if k < KLO else gHiS[:, k - KLO, :]

        for dn in range(DN):
            ds_ = slice(dn * DB, (dn + 1) * DB)
            quarters = quarters0 if dn == 0 else emit_w2quarters(dn)
            for tg in range(4):
                last = (dn == DN - 1) and (tg == 3)
                pss = [
                    psum.tile([P, DB], F32, tag="ps", name=f"pso_{dn}_{tg}_{i}")
                    for i in range(4)
                ]
                for ki, k in enumerate(ks_order):
                    wmv = quarters[ki // 8][:, ki % 8, :]
                    for i in range(4):
                        tm = tg * 4 + i
                        nc.tensor.matmul(
                            pss[i][:],
                            gblk(k)[:, tm * P:(tm + 1) * P],
                            wmv,
                            start=(ki == 0), stop=(ki == KH - 1),
                        )
                for i in range(4):
                    tm = tg * 4 + i
                    ev = oevict.tile([P, DB], F32, tag="ev")
                    if last:
                        # evictions split across both engines (halves
                        # the copy latency); one DMA per block — extra
                        # dispatches cost more than they save
                        nc.vector.tensor_copy(ev[:, 0:DB // 2], pss[i][:, 0:DB // 2])
                        nc.scalar.copy(ev[:, DB // 2:], pss[i][:, DB // 2:])
                    elif i % 2 == 0:
                        nc.vector.tensor_copy(ev[:], pss[i][:])
                    else:
                        nc.scalar.copy(ev[:], pss[i][:])
                    if i % 2 == 0:
                        nc.sync.dma_start(out[tm * P:(tm + 1) * P, ds_], ev[:])
                    else:
                        nc.scalar.dma_start(out[tm * P:(tm + 1) * P, ds_], ev[:])


def _build(num_devices=E):
    nc = bacc.Bacc("TRN2", debug=False, num_devices=num_devices)
    x = nc.dram_tensor("x", (T, D), F32, kind="ExternalInput").ap()
    w1 = nc.dram_tensor("w1", (D, H), F32, kind="ExternalInput").ap()
    w2 = nc.dram_tensor("w2", (H, D), F32, kind="ExternalInput").ap()
    w3 = nc.dram_tensor("w3", (D, H), F32, kind="ExternalInput").ap()
    out = nc.dram_tensor("out", (T, D), F32, kind="ExternalOutput").ap()
    gLoD = nc.dram_tensor("gLoD", (KLO * P, T), F16, kind="Internal").ap()
    with tile.TileContext(nc) as tc:
        _swiglu_body(tc, out, x, w1, w2, w3, gLoD)
    nc.compile()
    return nc


def _get_nc():
    global _CACHED_NC
    if _CACHED_NC is None:
        _CACHED_NC = _build()
    return _CACHED_NC


def kernel(x, w1, w2, w3):
    global LAST_RESULTS
    x = np.ascontiguousarray(np.asarray(x, dtype=np.float32))
    w1 = np.ascontiguousarray(np.asarray(w1, dtype=np.float32))
    w2 = np.ascontiguousarray(np.asarray(w2, dtype=np.float32))
    w3 = np.ascontiguousarray(np.asarray(w3, dtype=np.float32))
    assert x.shape == (E, T, D), x.shape

    nc = _get_nc()
    in_maps = [
        {"x": x[e], "w1": w1[e], "w2": w2[e], "w3": w3[e]} for e in range(E)
    ]
    res = run_bass_kernel_spmd(
        nc, in_maps, core_ids=list(range(E)), trace=TRACE
    )
    LAST_RESULTS = res
    return np.stack([res.results[e]["out"] for e in range(E)], axis=0)
